# revision 1
# baseline (speedup 1.0000x reference)
"""Distributed BasicGCN kernel for one Trainium2 chip (8 NeuronCores).

Strategy (graph/data parallel, hardcoded for N=50000 nodes / E=800000 edges):
  - Nodes are partitioned contiguously across the 8 cores (6250 each, padded
    to 6272 = 49*128). Node features h live on-chip in feature-major layout
    (hT: [128 feat x 2 halves, 6272 node cols], fp16).
  - Per GCN layer, each core computes g = (h @ W) * dinv for its node shard
    (TensorE), writes g (node-major fp16) to DRAM; two AllGathers replicate
    g across cores, split by local row window (A: rows [0,3200), B: rows
    [3200,6272)) so gather indices fit in int16 (8*3200=25600 rows max).
  - Edges are partitioned by destination. The weighted scatter-sum
    agg[d] = sum_e w_e * g[src_e] is computed as one-hot matmuls: gather 128
    source rows per chunk (SWDGE dma_gather, round-robined over 4 SWDGE
    queues - 4 queues lift gather throughput from ~70 to ~370 GB/s/core),
    then PE-matmul with a DVE-generated one-hot weight matrix [128 edges,
    64 dst] accumulated into PSUM [128 feat, 64 dst] (feature-major).
  - The scatter phase is split: first all tiles consume their A-window
    chunks (only needs AllGather A, which fires mid-phase-1), partial sums
    are evicted to SBUF fp16; then all tiles consume B-window chunks and
    combine. This hides most of AllGather B behind the A-pass.
  - One-hot tiles are generated on-device by DVE from compact per-group
    (dst-offset, weight) vectors: oh = (iota == dloc) * w, saving ~14 MB of
    HBM reads per layer.
  - Self-loops are folded in as ordinary edges with weight dinv[d]; the
    symmetric norm dinv[s]*ew*dinv[d] is split as (g absorbs dinv[s],
    one-hot weight absorbs ew*dinv[d]).
  - Embedding and decode layers are plain sharded matmuls.

All host-side preprocessing (degree/norm computation, edge binning, one-hot
metadata) is numpy; the device program structure is identical across cores
(required by SPMD), with per-core data shipped via in_maps.
"""

import sys

sys.path.insert(0, "/opt/trn_rl_repo")

import numpy as np

# ---------------------------------------------------------------- constants
NC = 8
N_NODES = 50000
IN_FEAT = 7
INPUT_SIZE = 12
DIN = IN_FEAT * INPUT_SIZE  # 84
HID = 256
OUT_FEAT = 7
FH = 24
DOUT = OUT_FEAT * FH  # 168
N_LAYERS = 5

PER = N_NODES // NC  # 6250 real nodes per core
NT128 = 49  # node tiles of 128 per core
PERP = NT128 * 128  # 6272 padded nodes per core
D = 64  # destination-tile size for the scatter matmul
NT64 = PERP // D  # 98 dst tiles per core

ALOC = 3200  # local rows [0, ALOC) are in gather buffer A
BLO = 3200  # local rows [BLO, PERP) are in gather buffer B
BPER = PERP - BLO  # 3072 rows per core in buffer B
AROWS = NC * ALOC  # 25600
BROWS = NC * BPER  # 24576

MAXCH = 14  # 128-edge chunks per dma_gather call
NQ = 4  # SWDGE queues for gather round-robin (1 queue: ~70 GB/s, 4: ~370)
MSG_BUFS = 6  # gather-call tiles in flight per stream
MSG_MERGED = 10  # >0: single shared msg pool with this many bufs
OH_MODE = "dve"  # "dve": per-group DVE gen; "dram": prebuild once to DRAM, slab loads
EVICT_ENG = "act"  # engine for A-pass psum eviction: "dve" | "act"
OHG = 32  # groups per one-hot DRAM slab (dram mode)
OH_BUFS = 16  # one-hot tiles of DVE run-ahead (dve mode)

ABLATE = set()  # dev-only: {"gather", "scatter_mm", "oh", "ag", "gphase"}

F16 = np.float16
F32 = np.float32


def _cdiv(a, b):
    return -(-a // b)


# ------------------------------------------------------------ host prep
def _prep(edge_index, edge_weights):
    """Bin edges by (core, dst-tile, gather-buffer), build per-core index and
    one-hot metadata plus the (uniform) program structure."""
    src = np.asarray(edge_index[0], dtype=np.int64)
    dst = np.asarray(edge_index[1], dtype=np.int64)
    ew = np.asarray(edge_weights, dtype=F32)

    deg = np.bincount(dst, weights=ew.astype(np.float64), minlength=N_NODES).astype(
        F32
    ) + F32(1.0)
    dinv = (1.0 / np.sqrt(deg)).astype(F32)

    # edges + self loops; one-hot weight = ew * dinv[dst] (self: dinv[d])
    allsrc = np.concatenate([src, np.arange(N_NODES, dtype=np.int64)])
    alldst = np.concatenate([dst, np.arange(N_NODES, dtype=np.int64)])
    allw = np.concatenate([ew * dinv[dst], dinv]).astype(F32)

    core_d = alldst // PER
    dl = alldst % PER
    t98 = dl // D
    gt = core_d * NT64 + t98  # global tile id
    dloc = dl % D
    core_s = allsrc // PER
    sl = allsrc % PER
    arow = (core_s * ALOC + sl).astype(np.int64)  # valid iff sl < ALOC
    brow = (core_s * BPER + (sl - BLO)).astype(np.int64)  # valid iff sl >= BLO
    cls = np.where(sl < BLO, 0, 2).astype(np.int64)

    NT = NC * NT64
    cnt = np.bincount(gt * 3 + cls, minlength=NT * 3).reshape(NC, NT64, 3)
    nAf, nfl, nBf = cnt[..., 0], cnt[..., 1], cnt[..., 2]
    ntot = nAf + nfl + nBf
    cA = _cdiv(nAf, 128).max(0)  # [NT64] uniform chunk counts, A stream
    SA = cA * 128
    nA = np.minimum(nAf + nfl, SA[None, :])  # per (core, tile) edges routed to A
    nB = ntot - nA
    cB = _cdiv(nB, 128).max(0)
    assert cA.min() > 0 and cB.min() > 0

    AoffC = np.concatenate([[0], np.cumsum(cA)])  # chunk offset of tile t in A stream
    BoffC = np.concatenate([[0], np.cumsum(cB)])
    NAc = int(AoffC[-1])
    NBc = int(BoffC[-1])
    gb = np.concatenate([[0], np.cumsum(cA + cB)])  # group base per tile
    Gtot = int(gb[-1])

    # order edges by (tile, class) so A-selected = first nA of each tile seg
    order = np.lexsort((allsrc, cls, gt))
    gts = gt[order]
    seg_start = np.searchsorted(gts, np.arange(NT))
    r = np.arange(len(order)) - seg_start[gts]  # rank within tile segment
    oc = gts // NT64
    ot = gts % NT64
    isA = r < nA[oc, ot]
    rb = r - nA[oc, ot]

    osrcA = arow[order]
    osrcB = brow[order]
    ow = allw[order]
    odloc = dloc[order]

    # stream positions
    posA = AoffC[ot] * 128 + r  # valid where isA
    posB = BoffC[ot] * 128 + rb  # valid where ~isA
    grp = np.where(isA, gb[ot] + r // 128, gb[ot] + cA[ot] + rb // 128)
    pos128 = np.where(isA, r % 128, rb % 128)

    idxA = np.zeros((NC, max(NAc, 1) * 128), np.int16)
    idxB = np.zeros((NC, max(NBc, 1) * 128), np.int16)
    a = isA
    b = ~isA
    idxA[oc[a], posA[a]] = osrcA[a].astype(np.int16)
    idxB[oc[b], posB[b]] = osrcB[b].astype(np.int16)

    # compact one-hot metadata: oh[p, g*64 + c] = w[p,g] * (c == dloc[p,g])
    dloc_arr = np.zeros((NC, 128, Gtot), F32)
    w_arr = np.zeros((NC, 128, Gtot), F32)
    dloc_arr[oc, pos128, grp] = odloc.astype(F32)
    w_arr[oc, pos128, grp] = ow.astype(F16).astype(F32)

    # sanity: all used indices in range
    assert osrcA[a].max(initial=0) < AROWS and osrcA[a].min(initial=0) >= 0
    assert osrcB[b].max(initial=0) < BROWS and osrcB[b].min(initial=0) >= 0

    def wrap(idx, nchunks):
        # [NC, n*128] -> [NC, 128, n*8]: position i lives at (i%16 + 16k, i//16)
        w16 = idx.reshape(NC, nchunks * 8, 16).transpose(0, 2, 1)
        return np.ascontiguousarray(np.tile(w16, (1, 8, 1)))

    idxA_w = wrap(idxA, max(NAc, 1))
    idxB_w = wrap(idxB, max(NBc, 1))

    # per-core dinv, padded
    dinv_pad = np.zeros((NC, PERP), F32)
    dinv_pad[:, :PER] = dinv.reshape(NC, PER)
    dinv_sb = np.ascontiguousarray(dinv_pad.reshape(NC, NT128, 128).transpose(0, 2, 1))

    iota = np.ascontiguousarray(
        np.broadcast_to(np.arange(D, dtype=F16), (128, D))
    )

    return dict(
        dinv=dinv,
        cA=cA,
        cB=cB,
        AoffC=AoffC,
        BoffC=BoffC,
        NAc=NAc,
        NBc=NBc,
        gb=gb,
        Gtot=Gtot,
        idxA_w=idxA_w,
        idxB_w=idxB_w,
        dloc=dloc_arr,
        ohw=w_arr,
        iota=iota,
        dinv_sb=dinv_sb,
        idxA_flat=idxA,
        idxB_flat=idxB,
    )


def _pack_weights(W_emb, b_emb, Wg, bg, W_dec, b_dec):
    """Pack weight matrices into the on-device layouts (shared across cores)."""
    Wemb_p = np.ascontiguousarray(W_emb.astype(F16))  # [84, 256]
    bemb_p = np.ascontiguousarray(
        b_emb.astype(F32).reshape(2, 128).T
    )  # [128, 2] (col h = bias[h*128+p])
    # Wg packed [128, 5*2*256]: layer l slab k -> cols [(l*2+k)*256, ...)
    Wg_p = np.zeros((128, N_LAYERS * 2 * HID), F16)
    for l in range(N_LAYERS):
        for k in range(2):
            Wg_p[:, (l * 2 + k) * HID : (l * 2 + k + 1) * HID] = Wg[l][
                k * 128 : (k + 1) * 128, :
            ].astype(F16)
    bg_p = np.ascontiguousarray(
        bg.astype(F32).reshape(N_LAYERS, 2, 128).transpose(2, 0, 1).reshape(128, -1)
    )  # [128, 5*2]: col l*2+h
    Wdec_p = np.zeros((128, 2 * DOUT), F16)
    for k in range(2):
        Wdec_p[:, k * DOUT : (k + 1) * DOUT] = W_dec[k * 128 : (k + 1) * 128, :].astype(
            F16
        )
    bdec_p = np.ascontiguousarray(np.broadcast_to(b_dec.astype(F32), (128, DOUT)))
    return Wemb_p, bemb_p, Wg_p, bg_p, Wdec_p, bdec_p


def _prep_x(x):
    """x [50000, 7, 12] -> per-core xT [84, 6272] fp16 (feature-major)."""
    xf = x.reshape(N_NODES, DIN).astype(F16)
    xT = np.zeros((NC, DIN, PERP), F16)
    for c in range(NC):
        xT[c, :, :PER] = xf[c * PER : (c + 1) * PER].T
    return xT


# ------------------------------------------------------------ device build
def _build(st):
    import concourse.bass as bass
    import concourse.mybir as mybir
    from concourse import tile, bacc

    f16 = mybir.dt.float16
    f32 = mybir.dt.float32
    i16 = mybir.dt.int16
    Relu = mybir.ActivationFunctionType.Relu
    IsEq = mybir.AluOpType.is_equal
    Mult = mybir.AluOpType.mult
    Add = mybir.AluOpType.add
    core_ids = list(range(NC))

    cA, cB = st["cA"], st["cB"]
    AoffC, BoffC = st["AoffC"], st["BoffC"]
    NAc, NBc, gb = st["NAc"], st["NBc"], st["gb"]
    Gtot = st["Gtot"]

    nc = bacc.Bacc(
        "TRN2",
        target_bir_lowering=False,
        debug=False,
        num_devices=NC,
        num_swdge_queues=NQ,
    )

    xT_d = nc.dram_tensor("xT", [DIN, PERP], f16, kind="ExternalInput")
    dinv_d = nc.dram_tensor("dinv", [128, NT128], f32, kind="ExternalInput")
    Wemb_d = nc.dram_tensor("Wemb", [DIN, HID], f16, kind="ExternalInput")
    bemb_d = nc.dram_tensor("bemb", [128, 2], f32, kind="ExternalInput")
    Wg_d = nc.dram_tensor("Wg", [128, N_LAYERS * 2 * HID], f16, kind="ExternalInput")
    bg_d = nc.dram_tensor("bg", [128, N_LAYERS * 2], f32, kind="ExternalInput")
    Wdec_d = nc.dram_tensor("Wdec", [128, 2 * DOUT], f16, kind="ExternalInput")
    bdec_d = nc.dram_tensor("bdec", [128, DOUT], f32, kind="ExternalInput")
    idxA_d = nc.dram_tensor("idxA", [128, max(NAc, 1) * 8], i16, kind="ExternalInput")
    idxB_d = nc.dram_tensor("idxB", [128, max(NBc, 1) * 8], i16, kind="ExternalInput")
    dloc_d = nc.dram_tensor("dloc", [128, Gtot], f32, kind="ExternalInput")
    ohw_d = nc.dram_tensor("ohw", [128, Gtot], f32, kind="ExternalInput")
    iota_d = nc.dram_tensor("iota", [128, D], f16, kind="ExternalInput")
    out_d = nc.dram_tensor("out", [PERP, DOUT], f32, kind="ExternalOutput")

    with tile.TileContext(nc) as tc:
        with (
            tc.tile_pool(name="const", bufs=1) as constp,
            tc.tile_pool(name="hbuf", bufs=1) as hbuf,
            tc.tile_pool(
                name="msgA", bufs=(MSG_MERGED if MSG_MERGED else MSG_BUFS)
            ) as msgAp,
            tc.tile_pool(
                name="msgB", bufs=(1 if MSG_MERGED else MSG_BUFS)
            ) as msgBp,
            tc.tile_pool(name="ohp", bufs=OH_BUFS if OH_MODE == "dve" else 4) as ohp,
            tc.tile_pool(name="gstg", bufs=2) as gstg,
            tc.tile_pool(name="ostg", bufs=2) as ostg,
            tc.tile_pool(name="tmpp", bufs=4) as tmpp,
            tc.tile_pool(name="psg", bufs=2, space="PSUM") as psg,
            tc.tile_pool(name="pss", bufs=3, space="PSUM") as pss,
            tc.tile_pool(name="dram", bufs=1, space="DRAM") as dramp,
            tc.tile_pool(name="dramg", bufs=4, space="DRAM") as dramgp,
        ):
            # ---- persistent DRAM tiles
            g_local = dramp.tile([PERP, HID], f16, tag="g_local")

            # ---- constants into SBUF
            def load_const(dram_t, shape, dtype, tag):
                t = constp.tile(shape, dtype, tag=tag)
                nc.sync.dma_start(t[:], dram_t[:])
                return t

            xT_sb = load_const(xT_d, [DIN, PERP], f16, "xT")
            dinv_sb = load_const(dinv_d, [128, NT128], f32, "dinv")
            Wemb_sb = load_const(Wemb_d, [DIN, HID], f16, "Wemb")
            bemb_sb = load_const(bemb_d, [128, 2], f32, "bemb")
            Wg_sb = load_const(Wg_d, [128, N_LAYERS * 2 * HID], f16, "Wg")
            bg_sb = load_const(bg_d, [128, N_LAYERS * 2], f32, "bg")
            Wdec_sb = load_const(Wdec_d, [128, 2 * DOUT], f16, "Wdec")
            bdec_sb = load_const(bdec_d, [128, DOUT], f32, "bdec")
            idxA_sb = load_const(idxA_d, [128, max(NAc, 1) * 8], i16, "idxA")
            idxB_sb = load_const(idxB_d, [128, max(NBc, 1) * 8], i16, "idxB")
            dloc_sb = load_const(dloc_d, [128, Gtot], f32, "dloc")
            ohw_sb = load_const(ohw_d, [128, Gtot], f32, "ohw")
            iota_sb = load_const(iota_d, [128, D], f16, "iota")

            hA = hbuf.tile([128, 2, PERP], f16, tag="hA")
            hB = hbuf.tile([128, 2, PERP], f16, tag="hB")

            NSLAB = _cdiv(Gtot, OHG)
            oh_dram = None
            if OH_MODE == "dram" and "oh" not in ABLATE:
                oh_dram = dramp.tile([128, NSLAB * OHG * D], f16, tag="oh_dram")
                for s in range(NSLAB):
                    slab = gstg.tile([128, 8, HID], f16, tag="gstg")
                    svw = slab[:].rearrange("p j f -> p (j f)")
                    ngs = min(OHG, Gtot - s * OHG)
                    for j in range(ngs):
                        g = s * OHG + j
                        nc.vector.tensor_scalar(
                            svw[:, j * D : (j + 1) * D],
                            iota_sb[:],
                            dloc_sb[:, g : g + 1],
                            ohw_sb[:, g : g + 1],
                            IsEq,
                            Mult,
                        )
                    if ngs < OHG:
                        nc.vector.memset(svw[:, ngs * D : OHG * D], 0.0)
                    nc.sync.dma_start(
                        oh_dram[:, s * OHG * D : (s + 1) * OHG * D],
                        svw[:, 0 : OHG * D],
                    )

            # ---- embedding: hA[:, k, t*128:...] = relu(Wemb[:,k].T @ xT + b)
            for t in range(NT128):
                cols = slice(t * 128, (t + 1) * 128)
                pse0 = psg.tile([128, 128], f32, tag="gps")
                nc.tensor.matmul(
                    pse0[:], Wemb_sb[:, 0:128], xT_sb[:, cols], start=True, stop=True
                )
                nc.scalar.activation(
                    hA[:, 0, cols], pse0[:], Relu, bias=bemb_sb[:, 0:1], scale=1.0
                )
                pse1 = psg.tile([128, 128], f32, tag="gps")
                nc.tensor.matmul(
                    pse1[:], Wemb_sb[:, 128:256], xT_sb[:, cols], start=True, stop=True
                )
                nc.scalar.activation(
                    hA[:, 1, cols], pse1[:], Relu, bias=bemb_sb[:, 1:2], scale=1.0
                )

            # ---- GCN layers
            qctr = [0]  # global SWDGE queue round-robin

            hcur, hnext = hA, hB
            for l in range(N_LAYERS):
                g_fullA = dramgp.tile(
                    [AROWS, HID], f16, tag="g_fullA", addr_space="Shared"
                )
                g_fullB = dramgp.tile(
                    [BROWS, HID], f16, tag="g_fullB", addr_space="Shared"
                )
                # phase 1: g = (h @ Wg[l]) * dinv, node-major fp16 -> g_local
                for tb in range(0, NT128, 8):
                    nb = min(8, NT128 - tb)
                    stg = gstg.tile([128, 8, HID], f16, tag="gstg")
                    for j in range(nb):
                        if "gphase" in ABLATE:
                            break
                        t = tb + j
                        cols = slice(t * 128, (t + 1) * 128)
                        ps = psg.tile([128, HID], f32, tag="gps")
                        nc.tensor.matmul(
                            ps[:],
                            hcur[:, 0, cols],
                            Wg_sb[:, (l * 2 + 0) * HID : (l * 2 + 1) * HID],
                            start=True,
                            stop=False,
                        )
                        nc.tensor.matmul(
                            ps[:],
                            hcur[:, 1, cols],
                            Wg_sb[:, (l * 2 + 1) * HID : (l * 2 + 2) * HID],
                            start=False,
                            stop=True,
                        )
                        nc.vector.tensor_scalar(
                            stg[:, j, :],
                            ps[:],
                            dinv_sb[:, t : t + 1],
                            None,
                            Mult,
                        )
                    rows = slice(tb * 128, (tb + nb) * 128)
                    if "gphase" not in ABLATE:
                        nc.sync.dma_start(
                            g_local[rows, :].rearrange("(j p) f -> p j f", p=128),
                            stg[:, 0:nb, :],
                        )
                    # A-window rows [0, 3200) = tiles 0..24 done after block 3
                    if tb + nb == 32 and "ag" not in ABLATE:
                        nc.gpsimd.collective_compute(
                            "AllGather",
                            mybir.AluOpType.bypass,
                            replica_groups=[core_ids],
                            ins=[g_local[0:ALOC, :]],
                            outs=[g_fullA[:]],
                        )
                if "ag" not in ABLATE:
                    nc.gpsimd.collective_compute(
                        "AllGather",
                        mybir.AluOpType.bypass,
                        replica_groups=[core_ids],
                        ins=[g_local[BLO:PERP, :]],
                        outs=[g_fullB[:]],
                    )

                # phase 2: scatter-aggregate into hnext.
                callA = {}
                callB = {}
                ohts = {}

                def ensure_call(k, calls, pool, tag, src_full, idx_sb, nstream):
                    if k in calls:
                        return calls[k]
                    nch = min(MAXCH, nstream - k * MAXCH)
                    mt = pool.tile([128, MAXCH, HID], f16, tag=tag)
                    if "gather" in ABLATE:
                        nc.vector.memset(mt[:, 0:1, 0:16], 0.0)
                        calls[k] = mt
                        return mt
                    nc.gpsimd.dma_gather(
                        mt[:, 0:nch, :],
                        src_full[:],
                        idx_sb[:, k * MAXCH * 8 : (k * MAXCH + nch) * 8],
                        nch * 128,
                        nch * 128,
                        HID,
                        single_packet=False,
                        queue_num=qctr[0] % NQ,
                    )
                    qctr[0] += 1
                    calls[k] = mt
                    return mt

                def ensure_oh(g):
                    """Returns (tile, column offset) for group g's one-hot."""
                    if OH_MODE == "dram":
                        s = g // OHG
                        if s in ohts:
                            return ohts[s], (g % OHG) * D
                        t = ohp.tile([128, OHG * D], f16, tag="oh")
                        if "oh" in ABLATE:
                            nc.vector.memset(t[:, 0:16], 0.0)
                        else:
                            nc.sync.dma_start(
                                t[:], oh_dram[:, s * OHG * D : (s + 1) * OHG * D]
                            )
                        ohts[s] = t
                        return t, (g % OHG) * D
                    if g in ohts:
                        return ohts[g], 0
                    t = ohp.tile([128, D], f16, tag="oh")
                    if "oh" in ABLATE:
                        nc.vector.memset(t[:, 0:16], 0.0)
                    else:
                        nc.vector.tensor_scalar(
                            t[:],
                            iota_sb[:],
                            dloc_sb[:, g : g + 1],
                            ohw_sb[:, g : g + 1],
                            IsEq,
                            Mult,
                        )
                    ohts[g] = t
                    return t, 0

                # ---- A-pass: consume A-window chunks, evict partials to hnext
                for t in range(NT64):
                    ps0 = pss.tile([128, D], f32, tag="ps0")
                    ps1 = pss.tile([128, D], f32, tag="ps1")
                    ng = int(cA[t])
                    for gi in range(ng):
                        g = int(gb[t]) + gi
                        oht, oco = ensure_oh(g)
                        ch = int(AoffC[t]) + gi
                        mt = ensure_call(
                            ch // MAXCH, callA, msgAp, "msgA", g_fullA, idxA_sb, NAc
                        )
                        c = ch % MAXCH
                        st_, sp = (gi == 0), (gi == ng - 1)
                        if "scatter_mm" in ABLATE:
                            if gi > 0:
                                continue
                            sp = True
                        nc.tensor.matmul(
                            ps0[:], mt[:, c, 0:128], oht[:, oco : oco + D],
                            start=st_, stop=sp,
                        )
                        nc.tensor.matmul(
                            ps1[:], mt[:, c, 128:256], oht[:, oco : oco + D],
                            start=st_, stop=sp,
                        )
                    cols = slice(t * D, (t + 1) * D)
                    if EVICT_ENG == "act":
                        Copy = mybir.ActivationFunctionType.Copy
                        nc.scalar.activation(hnext[:, 0, cols], ps0[:], Copy)
                        nc.scalar.activation(hnext[:, 1, cols], ps1[:], Copy)
                    else:
                        nc.vector.tensor_copy(hnext[:, 0, cols], ps0[:])
                        nc.vector.tensor_copy(hnext[:, 1, cols], ps1[:])

                # ---- B-pass: consume B-window chunks, combine + bias + relu
                for t in range(NT64):
                    ps0 = pss.tile([128, D], f32, tag="ps0")
                    ps1 = pss.tile([128, D], f32, tag="ps1")
                    ng = int(cB[t])
                    for gi in range(ng):
                        g = int(gb[t]) + int(cA[t]) + gi
                        oht, oco = ensure_oh(g)
                        ch = int(BoffC[t]) + gi
                        mt = ensure_call(
                            ch // MAXCH,
                            callB,
                            (msgAp if MSG_MERGED else msgBp),
                            ("msgA" if MSG_MERGED else "msgB"),
                            g_fullB,
                            idxB_sb,
                            NBc,
                        )
                        c = ch % MAXCH
                        st_, sp = (gi == 0), (gi == ng - 1)
                        if "scatter_mm" in ABLATE:
                            if gi > 0:
                                continue
                            sp = True
                        nc.tensor.matmul(
                            ps0[:], mt[:, c, 0:128], oht[:, oco : oco + D],
                            start=st_, stop=sp,
                        )
                        nc.tensor.matmul(
                            ps1[:], mt[:, c, 128:256], oht[:, oco : oco + D],
                            start=st_, stop=sp,
                        )
                    cols = slice(t * D, (t + 1) * D)
                    tm0 = tmpp.tile([128, D], f32, tag="tm0")
                    tm1 = tmpp.tile([128, D], f32, tag="tm1")
                    nc.vector.tensor_tensor(tm0[:], ps0[:], hnext[:, 0, cols], Add)
                    nc.vector.tensor_tensor(tm1[:], ps1[:], hnext[:, 1, cols], Add)
                    nc.scalar.activation(
                        hnext[:, 0, cols],
                        tm0[:],
                        Relu,
                        bias=bg_sb[:, l * 2 : l * 2 + 1],
                        scale=1.0,
                    )
                    nc.scalar.activation(
                        hnext[:, 1, cols],
                        tm1[:],
                        Relu,
                        bias=bg_sb[:, l * 2 + 1 : l * 2 + 2],
                        scale=1.0,
                    )
                hcur, hnext = hnext, hcur

            # ---- decode: out[n, :] = h @ Wdec + bdec
            for tb in range(0, NT128, 4):
                nb = min(4, NT128 - tb)
                ot = ostg.tile([128, 4, DOUT], f32, tag="ostg")
                for j in range(nb):
                    t = tb + j
                    cols = slice(t * 128, (t + 1) * 128)
                    ps = psg.tile([128, DOUT], f32, tag="gps")
                    nc.tensor.matmul(
                        ps[:],
                        hcur[:, 0, cols],
                        Wdec_sb[:, 0:DOUT],
                        start=True,
                        stop=False,
                    )
                    nc.tensor.matmul(
                        ps[:],
                        hcur[:, 1, cols],
                        Wdec_sb[:, DOUT : 2 * DOUT],
                        start=False,
                        stop=True,
                    )
                    nc.vector.tensor_tensor(
                        ot[:, j, :], ps[:], bdec_sb[:], mybir.AluOpType.add
                    )
                rows = slice(tb * 128, (tb + nb) * 128)
                nc.sync.dma_start(
                    out_d[rows, :].rearrange("(j p) f -> p j f", p=128),
                    ot[:, 0:nb, :],
                )

    nc.compile()
    return nc


def _in_maps(st, packed, xT):
    Wemb_p, bemb_p, Wg_p, bg_p, Wdec_p, bdec_p = packed
    maps = []
    for c in range(NC):
        maps.append(
            {
                "xT": xT[c],
                "dinv": st["dinv_sb"][c],
                "Wemb": Wemb_p,
                "bemb": bemb_p,
                "Wg": Wg_p,
                "bg": bg_p,
                "Wdec": Wdec_p,
                "bdec": bdec_p,
                "idxA": st["idxA_w"][c],
                "idxB": st["idxB_w"][c],
                "dloc": st["dloc"][c],
                "ohw": st["ohw"][c],
                "iota": st["iota"],
            }
        )
    return maps


# ------------------------------------------------------------ entry point
def kernel(x, edge_index, edge_weights, W_emb, b_emb, Wg, bg, W_dec, b_dec):
    from concourse.bass_utils import run_bass_kernel_spmd

    x = np.asarray(x)
    st = _prep(np.asarray(edge_index), np.asarray(edge_weights))
    packed = _pack_weights(
        np.asarray(W_emb),
        np.asarray(b_emb),
        np.asarray(Wg),
        np.asarray(bg),
        np.asarray(W_dec),
        np.asarray(b_dec),
    )
    xT = _prep_x(x)

    nc = _build(st)
    in_maps = _in_maps(st, packed, xT)

    res = run_bass_kernel_spmd(nc, in_maps, list(range(NC)))
    out = np.empty((N_NODES, DOUT), F32)
    for c in range(NC):
        out[c * PER : (c + 1) * PER] = res.results[c]["out"][:PER]
    return out.reshape(N_NODES, OUT_FEAT, FH)



# revision 2
# speedup vs baseline: 2.2705x; 2.2705x over previous
"""Distributed BasicGCN kernel for one Trainium2 chip (8 NeuronCores), v2.

Differences from v1:
  - Unit self-loops are never gathered: phase 1 additionally computes
    gT = (h @ W) in feature-major orientation and seeds
    hnext = gT * dinv^2 directly (DVE), so streams carry only real edges.
  - Edge streams are padded per (core, dst-tile64, window) to the max count
    over cores (not rounded up to 128); chunks of 128 edges may span dst-tile
    boundaries, with one one-hot matmul per (chunk, tile-segment). This keeps
    the SPMD program uniform while cutting gather padding from ~18% to ~6%.
  - Gather keeps 4 SWDGE queues busy with a deep msg pool and a large
    one-hot run-ahead pool (the v1 bottleneck was consumer-chain stalls
    limiting effective queue concurrency).
"""

import sys

sys.path.insert(0, "/opt/trn_rl_repo")

import numpy as np

# ---------------------------------------------------------------- constants
NC = 8
N_NODES = 50000
IN_FEAT = 7
INPUT_SIZE = 12
DIN = IN_FEAT * INPUT_SIZE  # 84
HID = 256
OUT_FEAT = 7
FH = 24
DOUT = OUT_FEAT * FH  # 168
N_LAYERS = 5

PER = N_NODES // NC  # 6250 real nodes per core
NT128 = 49  # node tiles of 128 per core
PERP = NT128 * 128  # 6272 padded nodes per core
D = 64  # destination-tile size for the scatter matmul
NT64 = PERP // D  # 98 dst tiles per core

ALOC = 3200  # local rows [0, ALOC) are in gather buffer A
BLO = 3200
BPER = PERP - BLO  # 3072
AROWS = NC * ALOC  # 25600
BROWS = NC * BPER  # 24576

MAXCH = 14  # 128-edge chunks per dma_gather call
NQ = 4
MSG_BUFS = 9
OH_BUFS = 64
PS_BUFS = 6
OH_MODE = "host"  # "host": precomputed one-hots streamed from DRAM; "dve": on-device gen
OHG = 32  # groups per one-hot slab (host mode)

ABLATE = set()  # {"gather", "scatter_mm", "oh", "ag", "gphase"}

F16 = np.float16
F32 = np.float32


def _cdiv(a, b):
    return -(-a // b)


# ------------------------------------------------------------ host prep
def _prep(edge_index, edge_weights):
    """Build per-core padded edge streams with chunk-spanning groups."""
    src = np.asarray(edge_index[0], dtype=np.int64)
    dst = np.asarray(edge_index[1], dtype=np.int64)
    ew = np.asarray(edge_weights, dtype=F32)

    deg = np.bincount(dst, weights=ew.astype(np.float64), minlength=N_NODES).astype(
        F32
    ) + F32(1.0)
    dinv = (1.0 / np.sqrt(deg)).astype(F32)

    # stream edges: real edges only; weight = ew * dinv[dst] (g absorbs dinv[src])
    allw = (ew * dinv[dst]).astype(F32)

    core_d = dst // PER
    dl = dst % PER
    t64 = dl // D
    dloc = dl % D
    core_s = src // PER
    sl = src % PER
    win = (sl >= BLO).astype(np.int64)  # 0 = A, 1 = B
    grow = np.where(win == 0, core_s * ALOC + sl, core_s * BPER + (sl - BLO))

    # per (core, tile, window) counts -> uniform padded counts N_ts
    cnt = np.zeros((NC, NT64, 2), np.int64)
    np.add.at(cnt, (core_d, t64, win), 1)
    Nts = cnt.max(axis=0)  # [NT64, 2]
    # every tile needs >=1 B group (the B pass applies bias+relu at stop)
    Nts[:, 1] = np.maximum(Nts[:, 1], 1)

    # stream layout per window: tile t occupies rows [P[t], P[t] + Nts[t])
    P = np.zeros((NT64, 2), np.int64)
    P[1:, 0] = np.cumsum(Nts[:-1, 0])
    P[1:, 1] = np.cumsum(Nts[:-1, 1])
    Ltot = [int(P[-1, w] + Nts[-1, w]) for w in range(2)]
    NCH = [_cdiv(Ltot[w], 128) for w in range(2)]  # chunks per stream
    Lpad = [NCH[w] * 128 for w in range(2)]

    # order edges into stream positions: sort by (core, win, t64, src row)
    order = np.lexsort((grow, t64, win, core_d))
    oc = core_d[order]
    ot = t64[order]
    ow_ = win[order]
    # rank within (core, win, t64) — key order must match the lexsort order
    key = (oc * 2 + ow_) * NT64 + ot
    ks = np.sort(key)
    seg_start = np.searchsorted(ks, np.arange(NC * 2 * NT64))
    rank = np.arange(len(order)) - seg_start[key]
    pos = P[ot, ow_] + rank  # stream position within (core, win)

    # build index + metadata arrays
    idx = [np.zeros((NC, Lpad[w]), np.int16) for w in range(2)]
    wv = [np.zeros((NC, Lpad[w]), F32) for w in range(2)]
    dv = [np.zeros((NC, Lpad[w]), F32) for w in range(2)]
    for w in range(2):
        m = ow_ == w
        idx[w][oc[m], pos[m]] = grow[order][m].astype(np.int16)
        wv[w][oc[m], pos[m]] = allw[order][m]
        dv[w][oc[m], pos[m]] = dloc[order][m].astype(F32)

    assert grow.max() < max(AROWS, BROWS) and grow.min() >= 0
    assert AROWS < 32768 and BROWS < 32768

    # group schedule per window: walk chunks, one group per (chunk, tile-seg)
    sched = []  # per window: list of (chunk, gcol, tile, start, stop)
    gcol = 0
    Gcols = []
    for w in range(2):
        lst = []
        t = 0
        for k in range(NCH[w]):
            c0, c1 = k * 128, (k + 1) * 128
            # advance to first tile overlapping this chunk
            while t < NT64 and P[t, w] + Nts[t, w] <= c0:
                t += 1
            tt = t
            while tt < NT64 and P[tt, w] < c1:
                lo = max(int(P[tt, w]), c0)
                hi = min(int(P[tt, w] + Nts[tt, w]), c1)
                if hi > lo:
                    start = P[tt, w] >= c0
                    stop = P[tt, w] + Nts[tt, w] <= c1
                    lst.append((k, gcol, tt, bool(start), bool(stop), lo - c0, hi - c0))
                    gcol += 1
                tt += 1
        sched.append(lst)
        Gcols.append(gcol)
    Gtot = gcol

    # metadata columns: dloc/ohw [128, Gtot]; zero weight outside [lo, hi)
    dloc_arr = np.zeros((NC, 128, Gtot), F32)
    ohw_arr = np.zeros((NC, 128, Gtot), F32)
    for w in range(2):
        for (k, g, tt, st, sp, lo, hi) in sched[w]:
            rows = slice(k * 128 + lo, k * 128 + hi)
            dloc_arr[:, lo:hi, g] = dv[w][:, rows]
            ohw_arr[:, lo:hi, g] = wv[w][:, rows].astype(F16).astype(F32)

    def wrap(ix, nchunks):
        w16 = ix.reshape(NC, nchunks * 8, 16).transpose(0, 2, 1)
        return np.ascontiguousarray(np.tile(w16, (1, 8, 1)))

    idxA_w = wrap(idx[0], NCH[0])
    idxB_w = wrap(idx[1], NCH[1])

    dinv_pad = np.zeros((NC, PERP), F32)
    dinv_pad[:, :PER] = dinv.reshape(NC, PER)
    dinv_sb = np.ascontiguousarray(dinv_pad.reshape(NC, NT128, 128).transpose(0, 2, 1))
    # dinv^2 broadcast across partitions, feature-major layout [128, PERP]
    dinv2b = np.ascontiguousarray(
        np.broadcast_to((dinv_pad**2).astype(F16)[:, None, :], (NC, 128, PERP))
    )

    iota = np.ascontiguousarray(np.broadcast_to(np.arange(D, dtype=F16), (128, D)))

    # host-precomputed one-hot slabs: oh[p, g*64 + c] = ohw[p,g] * (c == dloc[p,g])
    NSLAB = _cdiv(Gtot, OHG)
    Gpad = NSLAB * OHG
    eq = dloc_arr[..., None] == np.arange(D, dtype=F32)
    ohh = (eq * ohw_arr[..., None]).astype(F16)  # [NC, 128, Gtot, 64]
    oh_host = np.zeros((NC, 128, Gpad * D), F16)
    oh_host[:, :, : Gtot * D] = ohh.reshape(NC, 128, Gtot * D)

    return dict(
        oh_host=oh_host,
        dinv=dinv,
        NCH=NCH,
        sched=sched,
        Gtot=Gtot,
        idxA_w=idxA_w,
        idxB_w=idxB_w,
        dloc=dloc_arr,
        ohw=ohw_arr,
        iota=iota,
        dinv_sb=dinv_sb,
        dinv2b=dinv2b,
    )


def _pack_weights(W_emb, b_emb, Wg, bg, W_dec, b_dec):
    Wemb_p = np.ascontiguousarray(W_emb.astype(F16))  # [84, 256]
    bemb_p = np.ascontiguousarray(b_emb.astype(F32).reshape(2, 128).T)  # [128, 2]
    Wg_p = np.zeros((128, N_LAYERS * 2 * HID), F16)
    for l in range(N_LAYERS):
        for k in range(2):
            Wg_p[:, (l * 2 + k) * HID : (l * 2 + k + 1) * HID] = Wg[l][
                k * 128 : (k + 1) * 128, :
            ].astype(F16)
    bg_p = np.ascontiguousarray(
        bg.astype(F32).reshape(N_LAYERS, 2, 128).transpose(2, 0, 1).reshape(128, -1)
    )
    Wdec_p = np.zeros((128, 2 * DOUT), F16)
    for k in range(2):
        Wdec_p[:, k * DOUT : (k + 1) * DOUT] = W_dec[k * 128 : (k + 1) * 128, :].astype(
            F16
        )
    bdec_p = np.ascontiguousarray(np.broadcast_to(b_dec.astype(F32), (128, DOUT)))
    return Wemb_p, bemb_p, Wg_p, bg_p, Wdec_p, bdec_p


def _prep_x(x):
    xf = x.reshape(N_NODES, DIN).astype(F16)
    xT = np.zeros((NC, DIN, PERP), F16)
    for c in range(NC):
        xT[c, :, :PER] = xf[c * PER : (c + 1) * PER].T
    return xT


# ------------------------------------------------------------ device build
def _build(st):
    import concourse.bass as bass
    import concourse.mybir as mybir
    from concourse import tile, bacc

    f16 = mybir.dt.float16
    f32 = mybir.dt.float32
    i16 = mybir.dt.int16
    Relu = mybir.ActivationFunctionType.Relu
    IsEq = mybir.AluOpType.is_equal
    Mult = mybir.AluOpType.mult
    Add = mybir.AluOpType.add
    core_ids = list(range(NC))

    NCHA, NCHB = st["NCH"]
    schedA, schedB = st["sched"]
    Gtot = st["Gtot"]

    nc = bacc.Bacc(
        "TRN2",
        target_bir_lowering=False,
        debug=False,
        num_devices=NC,
        num_swdge_queues=NQ,
    )

    xT_d = nc.dram_tensor("xT", [DIN, PERP], f16, kind="ExternalInput")
    dinv_d = nc.dram_tensor("dinv", [128, NT128], f32, kind="ExternalInput")
    dinv2b_d = nc.dram_tensor("dinv2b", [128, PERP], f16, kind="ExternalInput")
    Wemb_d = nc.dram_tensor("Wemb", [DIN, HID], f16, kind="ExternalInput")
    bemb_d = nc.dram_tensor("bemb", [128, 2], f32, kind="ExternalInput")
    Wg_d = nc.dram_tensor("Wg", [128, N_LAYERS * 2 * HID], f16, kind="ExternalInput")
    bg_d = nc.dram_tensor("bg", [128, N_LAYERS * 2], f32, kind="ExternalInput")
    Wdec_d = nc.dram_tensor("Wdec", [128, 2 * DOUT], f16, kind="ExternalInput")
    bdec_d = nc.dram_tensor("bdec", [128, DOUT], f32, kind="ExternalInput")
    idxA_d = nc.dram_tensor("idxA", [128, NCHA * 8], i16, kind="ExternalInput")
    idxB_d = nc.dram_tensor("idxB", [128, NCHB * 8], i16, kind="ExternalInput")
    dloc_d = nc.dram_tensor("dloc", [128, Gtot], f32, kind="ExternalInput")
    ohw_d = nc.dram_tensor("ohw", [128, Gtot], f32, kind="ExternalInput")
    iota_d = nc.dram_tensor("iota", [128, D], f16, kind="ExternalInput")
    NSLAB = _cdiv(Gtot, OHG)
    oh_d = nc.dram_tensor("ohh", [128, NSLAB * OHG * D], f16, kind="ExternalInput")
    out_d = nc.dram_tensor("out", [PERP, DOUT], f32, kind="ExternalOutput")

    with tile.TileContext(nc) as tc:
        with (
            tc.tile_pool(name="const", bufs=1) as constp,
            tc.tile_pool(name="hbuf", bufs=1) as hbuf,
            tc.tile_pool(name="msgp", bufs=MSG_BUFS) as msgp,
            tc.tile_pool(name="ohp", bufs=(4 if OH_MODE == "host" else OH_BUFS)) as ohp,
            tc.tile_pool(name="gstg", bufs=2) as gstg,
            tc.tile_pool(name="ostg", bufs=2) as ostg,
            tc.tile_pool(name="tmpp", bufs=4) as tmpp,
            tc.tile_pool(name="psg", bufs=2, space="PSUM") as psg,
            tc.tile_pool(name="psT", bufs=2, space="PSUM") as psT,
            tc.tile_pool(name="pss0", bufs=2, space="PSUM") as pss0,
            tc.tile_pool(name="pss1", bufs=2, space="PSUM") as pss1,
            tc.tile_pool(name="dram", bufs=1, space="DRAM") as dramp,
            tc.tile_pool(name="dramg", bufs=4, space="DRAM") as dramgp,
        ):
            g_local = dramp.tile([PERP, HID], f16, tag="g_local")

            def load_const(dram_t, shape, dtype, tag):
                t = constp.tile(shape, dtype, tag=tag)
                nc.sync.dma_start(t[:], dram_t[:])
                return t

            xT_sb = load_const(xT_d, [DIN, PERP], f16, "xT")
            dinv_sb = load_const(dinv_d, [128, NT128], f32, "dinv")
            dinv2b_sb = load_const(dinv2b_d, [128, PERP], f16, "dinv2b")
            Wemb_sb = load_const(Wemb_d, [DIN, HID], f16, "Wemb")
            bemb_sb = load_const(bemb_d, [128, 2], f32, "bemb")
            Wg_sb = load_const(Wg_d, [128, N_LAYERS * 2 * HID], f16, "Wg")
            bg_sb = load_const(bg_d, [128, N_LAYERS * 2], f32, "bg")
            Wdec_sb = load_const(Wdec_d, [128, 2 * DOUT], f16, "Wdec")
            bdec_sb = load_const(bdec_d, [128, DOUT], f32, "bdec")
            idxA_sb = load_const(idxA_d, [128, NCHA * 8], i16, "idxA")
            idxB_sb = load_const(idxB_d, [128, NCHB * 8], i16, "idxB")
            if OH_MODE != "host":
                dloc_sb = load_const(dloc_d, [128, Gtot], f32, "dloc")
                ohw_sb = load_const(ohw_d, [128, Gtot], f32, "ohw")
            iota_sb = load_const(iota_d, [128, D], f16, "iota")

            hA = hbuf.tile([128, 2, PERP], f16, tag="hA")
            hB = hbuf.tile([128, 2, PERP], f16, tag="hB")

            # ---- embedding
            for t in range(NT128):
                cols = slice(t * 128, (t + 1) * 128)
                pse0 = psg.tile([128, 128], f32, tag="gps")
                nc.tensor.matmul(
                    pse0[:], Wemb_sb[:, 0:128], xT_sb[:, cols], start=True, stop=True
                )
                nc.scalar.activation(
                    hA[:, 0, cols], pse0[:], Relu, bias=bemb_sb[:, 0:1], scale=1.0
                )
                pse1 = psg.tile([128, 128], f32, tag="gps")
                nc.tensor.matmul(
                    pse1[:], Wemb_sb[:, 128:256], xT_sb[:, cols], start=True, stop=True
                )
                nc.scalar.activation(
                    hA[:, 1, cols], pse1[:], Relu, bias=bemb_sb[:, 1:2], scale=1.0
                )

            qctr = [0]

            hcur, hnext = hA, hB
            for l in range(N_LAYERS):
                g_fullA = dramgp.tile(
                    [AROWS, HID], f16, tag="g_fullA", addr_space="Shared"
                )
                g_fullB = dramgp.tile(
                    [BROWS, HID], f16, tag="g_fullB", addr_space="Shared"
                )
                # ---- phase 1: g = (h @ Wg) * dinv -> g_local (node-major);
                #      gT = (h @ Wg) feature-major; hnext = gT * dinv^2
                for tb in range(0, NT128, 8):
                    nb = min(8, NT128 - tb)
                    stg = gstg.tile([128, 8, HID], f16, tag="gstg")
                    for j in range(nb):
                        if "gphase" in ABLATE:
                            break
                        t = tb + j
                        cols = slice(t * 128, (t + 1) * 128)
                        ps = psg.tile([128, HID], f32, tag="gps")
                        nc.tensor.matmul(
                            ps[:],
                            hcur[:, 0, cols],
                            Wg_sb[:, (l * 2 + 0) * HID : (l * 2 + 1) * HID],
                            start=True,
                            stop=False,
                        )
                        nc.tensor.matmul(
                            ps[:],
                            hcur[:, 1, cols],
                            Wg_sb[:, (l * 2 + 1) * HID : (l * 2 + 2) * HID],
                            start=False,
                            stop=True,
                        )
                        nc.vector.tensor_scalar(
                            stg[:, j, :],
                            ps[:],
                            dinv_sb[:, t : t + 1],
                            None,
                            Mult,
                        )
                        # gT halves: gT[k] = sum_j Wg[j->k].T @ h[j]
                        for k in range(2):
                            pt = psT.tile([128, 128], f32, tag="gpt")
                            nc.tensor.matmul(
                                pt[:],
                                Wg_sb[
                                    :,
                                    (l * 2 + 0) * HID + k * 128 : (l * 2 + 0) * HID
                                    + (k + 1) * 128,
                                ],
                                hcur[:, 0, cols],
                                start=True,
                                stop=False,
                            )
                            nc.tensor.matmul(
                                pt[:],
                                Wg_sb[
                                    :,
                                    (l * 2 + 1) * HID + k * 128 : (l * 2 + 1) * HID
                                    + (k + 1) * 128,
                                ],
                                hcur[:, 1, cols],
                                start=False,
                                stop=True,
                            )
                            nc.vector.tensor_tensor(
                                hnext[:, k, cols], pt[:], dinv2b_sb[:, cols], Mult
                            )
                    rows = slice(tb * 128, (tb + nb) * 128)
                    if "gphase" not in ABLATE:
                        nc.sync.dma_start(
                            g_local[rows, :].rearrange("(j p) f -> p j f", p=128),
                            stg[:, 0:nb, :],
                        )
                    if tb + nb == 32 and "ag" not in ABLATE:
                        nc.gpsimd.collective_compute(
                            "AllGather",
                            mybir.AluOpType.bypass,
                            replica_groups=[core_ids],
                            ins=[g_local[0:ALOC, :]],
                            outs=[g_fullA[:]],
                        )
                if "ag" not in ABLATE:
                    nc.gpsimd.collective_compute(
                        "AllGather",
                        mybir.AluOpType.bypass,
                        replica_groups=[core_ids],
                        ins=[g_local[BLO:PERP, :]],
                        outs=[g_fullB[:]],
                    )

                # ---- scatter passes
                calls = {}

                def ensure_call(key, src_full, idx_sb, nstream):
                    if key in calls:
                        return calls[key]
                    ci = key[1]
                    nch = min(MAXCH, nstream - ci * MAXCH)
                    mt = msgp.tile([128, MAXCH, HID], f16, tag="msg")
                    if "gather" in ABLATE:
                        nc.vector.memset(mt[:, 0:1, 0:16], 0.0)
                        calls[key] = mt
                        return mt
                    nc.gpsimd.dma_gather(
                        mt[:, 0:nch, :],
                        src_full[:],
                        idx_sb[:, ci * MAXCH * 8 : (ci * MAXCH + nch) * 8],
                        nch * 128,
                        nch * 128,
                        HID,
                        single_packet=False,
                        queue_num=qctr[0] % NQ,
                    )
                    qctr[0] += 1
                    calls[key] = mt
                    return mt

                ohts = {}

                def gen_oh(g):
                    if OH_MODE == "host":
                        s = g // OHG
                        if s in ohts:
                            return ohts[s], (g % OHG) * D
                        t = ohp.tile([128, OHG * D], f16, tag="oh")
                        if "oh" in ABLATE:
                            nc.vector.memset(t[:, 0:16], 0.0)
                        else:
                            nc.sync.dma_start(
                                t[:], oh_d[:, s * OHG * D : (s + 1) * OHG * D]
                            )
                        ohts[s] = t
                        return t, (g % OHG) * D
                    t = ohp.tile([128, D], f16, tag="oh")
                    if "oh" in ABLATE:
                        nc.vector.memset(t[:, 0:16], 0.0)
                    else:
                        nc.vector.tensor_scalar(
                            t[:],
                            iota_sb[:],
                            dloc_sb[:, g : g + 1],
                            ohw_sb[:, g : g + 1],
                            IsEq,
                            Mult,
                        )
                    return t, 0

                def run_pass(sched, nstream, src_full, idx_sb, skey, is_b):
                    live = {}
                    for (k, g, tt, st_, sp, lo, hi) in sched:
                        mt = ensure_call((skey, k // MAXCH), src_full, idx_sb, nstream)
                        c = k % MAXCH
                        oht, oco = gen_oh(g)
                        if st_:
                            ps0_new = pss0.tile([128, D], f32, tag="ps0")
                            ps1_new = pss1.tile([128, D], f32, tag="ps1")
                            live[tt] = (ps0_new, ps1_new)
                        ps0, ps1 = live[tt]
                        do_mm = ("scatter_mm" not in ABLATE) or st_
                        if do_mm:
                            do_sp = sp or ("scatter_mm" in ABLATE)
                            nc.tensor.matmul(
                                ps0[:],
                                mt[:, c, 0:128],
                                oht[:, oco : oco + D],
                                start=st_,
                                stop=do_sp,
                            )
                            nc.tensor.matmul(
                                ps1[:],
                                mt[:, c, 128:256],
                                oht[:, oco : oco + D],
                                start=st_,
                                stop=do_sp,
                            )
                        if sp:
                            cols = slice(tt * D, (tt + 1) * D)
                            if not is_b:
                                # hnext += ps  (hnext holds the self-loop seed)
                                nc.vector.tensor_tensor(
                                    hnext[:, 0, cols], ps0[:], hnext[:, 0, cols], Add
                                )
                                nc.vector.tensor_tensor(
                                    hnext[:, 1, cols], ps1[:], hnext[:, 1, cols], Add
                                )
                            else:
                                tm0 = tmpp.tile([128, D], f32, tag="tm0")
                                tm1 = tmpp.tile([128, D], f32, tag="tm1")
                                nc.vector.tensor_tensor(
                                    tm0[:], ps0[:], hnext[:, 0, cols], Add
                                )
                                nc.vector.tensor_tensor(
                                    tm1[:], ps1[:], hnext[:, 1, cols], Add
                                )
                                nc.scalar.activation(
                                    hnext[:, 0, cols],
                                    tm0[:],
                                    Relu,
                                    bias=bg_sb[:, l * 2 : l * 2 + 1],
                                    scale=1.0,
                                )
                                nc.scalar.activation(
                                    hnext[:, 1, cols],
                                    tm1[:],
                                    Relu,
                                    bias=bg_sb[:, l * 2 + 1 : l * 2 + 2],
                                    scale=1.0,
                                )
                            del live[tt]

                run_pass(schedA, NCHA, g_fullA, idxA_sb, (l, 0), False)
                run_pass(schedB, NCHB, g_fullB, idxB_sb, (l, 1), True)
                hcur, hnext = hnext, hcur

            # ---- decode
            for tb in range(0, NT128, 4):
                nb = min(4, NT128 - tb)
                ot = ostg.tile([128, 4, DOUT], f32, tag="ostg")
                for j in range(nb):
                    t = tb + j
                    cols = slice(t * 128, (t + 1) * 128)
                    ps = psg.tile([128, DOUT], f32, tag="gps")
                    nc.tensor.matmul(
                        ps[:],
                        hcur[:, 0, cols],
                        Wdec_sb[:, 0:DOUT],
                        start=True,
                        stop=False,
                    )
                    nc.tensor.matmul(
                        ps[:],
                        hcur[:, 1, cols],
                        Wdec_sb[:, DOUT : 2 * DOUT],
                        start=False,
                        stop=True,
                    )
                    nc.vector.tensor_tensor(
                        ot[:, j, :], ps[:], bdec_sb[:], mybir.AluOpType.add
                    )
                rows = slice(tb * 128, (tb + nb) * 128)
                nc.sync.dma_start(
                    out_d[rows, :].rearrange("(j p) f -> p j f", p=128),
                    ot[:, 0:nb, :],
                )

    nc.compile()
    return nc


def _in_maps(st, packed, xT):
    Wemb_p, bemb_p, Wg_p, bg_p, Wdec_p, bdec_p = packed
    maps = []
    for c in range(NC):
        maps.append(
            {
                "xT": xT[c],
                "dinv": st["dinv_sb"][c],
                "dinv2b": st["dinv2b"][c],
                "Wemb": Wemb_p,
                "bemb": bemb_p,
                "Wg": Wg_p,
                "bg": bg_p,
                "Wdec": Wdec_p,
                "bdec": bdec_p,
                "idxA": st["idxA_w"][c],
                "idxB": st["idxB_w"][c],
                "dloc": st["dloc"][c],
                "ohw": st["ohw"][c],
                "iota": st["iota"],
                "ohh": st["oh_host"][c],
            }
        )
    return maps


# ------------------------------------------------------------ entry point
def kernel(x, edge_index, edge_weights, W_emb, b_emb, Wg, bg, W_dec, b_dec):
    from concourse.bass_utils import run_bass_kernel_spmd

    x = np.asarray(x)
    st = _prep(np.asarray(edge_index), np.asarray(edge_weights))
    packed = _pack_weights(
        np.asarray(W_emb),
        np.asarray(b_emb),
        np.asarray(Wg),
        np.asarray(bg),
        np.asarray(W_dec),
        np.asarray(b_dec),
    )
    xT = _prep_x(x)

    nc = _build(st)
    in_maps = _in_maps(st, packed, xT)

    res = run_bass_kernel_spmd(nc, in_maps, list(range(NC)))
    out = np.empty((N_NODES, DOUT), F32)
    for c in range(NC):
        out[c * PER : (c + 1) * PER] = res.results[c]["out"][:PER]
    return out.reshape(N_NODES, OUT_FEAT, FH)
